# revision 14
# baseline (speedup 1.0000x reference)
"""ConnectionProductBlock on 8 TRN2 NeuronCores.

out[b, c*K + k, h, w] = am_out[b, c, h, w] * first_out[b, k, h, w]
  with B=16, C=8, K=64, H=W=56.

Strategy (data parallel over batch, 2 batches per core, no communication):
  - All device traffic is bf16 (rel err ~1e-2 max-elementwise, ~3e-3 l2,
    under the 2e-2 gate): halves the HBM-bound output traffic vs fp32.
    Host converts inputs fp32->bf16 and the returned bf16 output -> fp32.
  - SBUF layout: channels on partitions, hw (=3136) on the free dim.
  - am needs a partition-broadcast (am[b, c] replicated across the 64 k
    partitions of batch b). Compute engines have fixed lane<->partition
    wiring, so the replication runs on the TensorEngine: a K=16 selector
    matmul sel_c.T @ am writes rep[p, f] = am[p//64, c, f] into PSUM
    (fp32) in one-bank 512-column chunks (PE measured at a steady
    1.2GHz = 0.83ns/col on this part, ~21us for the 8*3136 columns).
  - PSUM fp32 operands cap DVE tensor_tensor at 1 elem/cycle, so Act and
    DVE copy/convert rep into SBUF bf16 in 2-bank groups (amortizes the
    Act engine's ~280ns fixed access cost); the multiply first2 * rep_sb
    -> out_t then runs as all-bf16-SBUF tensor_tensor at 2 elem/cycle on
    DVE (2x_1p mode), with GPSIMD taking some channels to balance load.
  - Output DRAM layout is [b, k, c, hw] (host untransposes): out_t tiles
    hold channel PAIRS so each DMA partition row is a contiguous 12.5KB
    run instead of 6.3KB, amortizing per-packet DMA overhead.
HBM traffic per core is ~7.3MB (6.4MB out + 0.9MB in), the bf16 minimum.
"""

import numpy as np

B, C, K, H, W = 16, 8, 64, 56, 56
HW = H * W  # 3136
NCORES = 8
BPC = B // NCORES  # batches per core = 2
MM = 512  # one PSUM bank of fp32 per matmul
# Copy groups per c: three 2-bank groups (2x512) + one 64-col tail.
GROUPS = [(0, 1024), (1024, 1024), (2048, 1024), (3072, 64)]
CPAIR = 2  # channels per out tile / output DMA

_PROGRAMS = {}


def _build_program(
    repeat=1,
    do_compute=True,
    do_out_dma=True,
    dve_copy=((0, 0), (2, 0), (4, 0), (6, 0)),  # (c, group) copies on DVE + tails
    pool_mult=(),  # c's whose multiply runs on GPSIMD (6.5us each: too slow)
    pe_only=False,  # bench: matmuls only, no copies/mults
):
    """repeat>1 wraps the whole body in a hardware loop; bench-only.
    do_compute/do_out_dma isolate pipeline stages for benchmarking."""
    import contextlib

    import concourse.bacc as bacc
    import concourse.mybir as mybir
    import concourse.tile as tile

    nc = bacc.Bacc("TRN2", debug=False)
    # am data + per-c selector blocks on the free dim, one bf16 plane.
    # Partition = b*8 + c. One DMA covers data + selectors so each matmul
    # carries a single sem wait.
    amsel = nc.dram_tensor(
        "amsel", [BPC * C, HW + C * BPC * K], mybir.dt.bfloat16, kind="ExternalInput"
    )
    first = nc.dram_tensor(
        "first", [BPC * K, HW], mybir.dt.bfloat16, kind="ExternalInput"
    )
    # [b, k, c, hw]: per (b, k) partition the C channels are contiguous, so
    # a channel-pair DMA writes 2*HW*2B = 12.5KB runs. Host untransposes.
    out = nc.dram_tensor(
        "out", [BPC, K, C, HW], mybir.dt.bfloat16, kind="ExternalOutput"
    )

    with tile.TileContext(nc) as tc:
        with (
            tc.tile_pool(name="ins", bufs=1) as ins_pool,
            tc.tile_pool(name="rep", bufs=3, space="PSUM") as psum_pool,
            tc.tile_pool(name="rept", bufs=2, space="PSUM") as psumt_pool,
            tc.tile_pool(name="repsb", bufs=3) as repsb_pool,
            tc.tile_pool(name="outs", bufs=2) as out_pool,
            tc.For_i(0, repeat, 1) if repeat > 1 else contextlib.nullcontext(),
        ):
            # amsel loads first: the PE broadcast only needs amsel, so it
            # starts while first2 (4x bigger) is still in flight.
            am3 = ins_pool.tile([BPC * C, HW + C * BPC * K], mybir.dt.bfloat16)
            nc.sync.dma_start(out=am3[:], in_=amsel.ap())
            first2 = ins_pool.tile([BPC * K, HW], mybir.dt.bfloat16)
            nc.sync.dma_start(out=first2[:], in_=first.ap())

            out_ap = out.ap()
            out_t = None
            for c in range(C):
                half = c % CPAIR
                if half == 0:
                    out_t = out_pool.tile(
                        [BPC * K, CPAIR * HW], mybir.dt.bfloat16, tag="out"
                    )
                if do_compute:
                    rep_sb = repsb_pool.tile(
                        [BPC * K, HW], mybir.dt.bfloat16, tag="repsb"
                    )
                    for gi, (g0, gn) in enumerate(GROUPS):
                        if gn == 64:
                            rep = psumt_pool.tile(
                                [BPC * K, 64], mybir.dt.float32, tag="rt", name="rep_t"
                            )
                        else:
                            rep = psum_pool.tile(
                                [BPC * K, 1024], mybir.dt.float32, tag="rep", name="rep"
                            )
                        for m0 in range(0, gn, MM):
                            mn = min(MM, gn - m0)
                            nc.tensor.matmul(
                                rep[:, m0 : m0 + mn],
                                lhsT=am3[
                                    :, HW + c * BPC * K : HW + (c + 1) * BPC * K
                                ],
                                rhs=am3[:, g0 + m0 : g0 + m0 + mn],
                                start=True,
                                stop=True,
                            )
                        # PSUM fp32 -> SBUF bf16 convert-copy (Act/DVE split)
                        if pe_only:
                            continue
                        if gn == 64 or (c, gi) in dve_copy:
                            nc.vector.tensor_copy(
                                rep_sb[:, g0 : g0 + gn], rep[:, 0:gn]
                            )
                        else:
                            nc.scalar.copy(rep_sb[:, g0 : g0 + gn], rep[:, 0:gn])
                    # all-bf16 SBUF tensor_tensor: 2x_1p on DVE; GPSIMD takes
                    # some c's to balance engine load.
                    if not pe_only:
                        eng = nc.gpsimd if c in pool_mult else nc.vector
                        if c >= C - CPAIR:
                            # Last pair: multiply in halves so each half's
                            # DMA ships while the next half still computes.
                            hh = HW // 2
                            for q in range(2):
                                eng.tensor_mul(
                                    out_t[
                                        :, half * HW + q * hh : half * HW + (q + 1) * hh
                                    ],
                                    first2[:, q * hh : (q + 1) * hh],
                                    rep_sb[:, q * hh : (q + 1) * hh],
                                )
                        else:
                            eng.tensor_mul(
                                out_t[:, half * HW : (half + 1) * HW],
                                first2[:],
                                rep_sb[:],
                            )
                else:
                    nc.vector.memset(out_t[:, half * HW + 0 : half * HW + 2], 0.0)
                if do_out_dma:
                    cp = c - half
                    if c >= C - CPAIR:
                        # Last pair: per-(c, hw-half, b) DMAs spread across
                        # the SP/Act/Pool DGE rings so the drain overlaps the
                        # remaining multiplies instead of serializing on SP.
                        hh = HW // 2
                        rings = [nc.sync, nc.scalar, nc.gpsimd, nc.sync]
                        for q in range(2):
                            for b in range(BPC):
                                rings[2 * q + b].dma_start(
                                    out=out_ap[
                                        b, :, c : c + 1, q * hh : (q + 1) * hh
                                    ],
                                    in_=out_t[
                                        b * K : (b + 1) * K,
                                        half * HW + q * hh : half * HW + (q + 1) * hh,
                                    ],
                                )
                    elif half == CPAIR - 1:
                        # One DMA per batch per channel pair: SBUF [64, 2*HW]
                        # -> DRAM [64, 2, HW] rows (k, c, hw) matching order.
                        for b in range(BPC):
                            nc.sync.dma_start(
                                out=out_ap[b, :, cp : cp + CPAIR, :],
                                in_=out_t[b * K : (b + 1) * K, :],
                            )
    nc.compile()
    return nc


def _get_program(repeat=1, **variant):
    key = (repeat, tuple(sorted(variant.items())))
    if key not in _PROGRAMS:
        _PROGRAMS[key] = _build_program(repeat, **variant)
    return _PROGRAMS[key]


def _make_sel():
    # One [16, 128] selector block per c: sel[b*C + c, c*128 + b*64 + k] = 1
    sel = np.zeros((BPC * C, C * BPC * K), dtype=np.float32)
    for c in range(C):
        for b in range(BPC):
            sel[b * C + c, c * BPC * K + b * K : c * BPC * K + (b + 1) * K] = 1.0
    return sel


def _make_amsel(am_core):
    """am_core [BPC*C, HW] fp32 -> [BPC*C, HW + 1024] bf16 with the per-c
    selector blocks appended on the free dim."""
    import ml_dtypes

    bf16 = ml_dtypes.bfloat16
    return np.ascontiguousarray(
        np.concatenate([am_core.astype(bf16), _make_sel().astype(bf16)], axis=1)
    )


def _run(am_np, first_np, variant=None, **spmd_kwargs):
    import ml_dtypes

    from concourse.bass_utils import run_bass_kernel_spmd

    bf16 = ml_dtypes.bfloat16
    nc = _get_program(**(variant or {}))
    in_maps = []
    for i in range(NCORES):
        am_i = am_np[BPC * i : BPC * (i + 1)].reshape(BPC * C, HW)
        first_i = first_np[BPC * i : BPC * (i + 1)].reshape(BPC * K, HW)
        in_maps.append(
            {
                "amsel": _make_amsel(am_i),
                "first": np.ascontiguousarray(first_i.astype(bf16)),
            }
        )
    return run_bass_kernel_spmd(nc, in_maps, core_ids=list(range(NCORES)), **spmd_kwargs)


def kernel(am_out, first_out):
    am_np = np.asarray(am_out, dtype=np.float32).reshape(B, C, HW)
    first_np = np.asarray(first_out, dtype=np.float32).reshape(B, K, HW)
    res = _run(am_np, first_np)
    # device layout is [b, k, c, hw] per core -> [b, c*K + k, hw]
    out = np.concatenate(
        [
            np.asarray(res.results[i]["out"], dtype=np.float32).transpose(0, 2, 1, 3)
            for i in range(NCORES)
        ],
        axis=0,
    )
    return np.ascontiguousarray(out.reshape(B, C * K, H, W))
